# revision 42
# baseline (speedup 1.0000x reference)
"""Mixtral MoE block (E=8 experts, top-2, H=1024, I=3584) on 8 Trainium2 cores.

Strategy (expert-parallel, per sharding hint):
  - Host computes the router (logits -> softmax -> top-2 -> renormalized
    combine weights). This is 0.04% of the model FLOPs; all heavy compute
    (the expert SwiGLU MLPs, ~90 GFLOP) runs on the NeuronCores.
  - Tokens are gathered per expert on the host (dispatch); core e computes
    expert e's MLP over a capacity-C batch of its tokens:
        hT = silu(W1 xT) * (W3 xT)                      [I, C]
        outT = W2-col-slices^T @ hT, scaled by routing weight   [H, C]
    (token-transposed layouts throughout; zero on-device transposes)
  - Host scatter-adds the per-core weighted outputs back into [T, H]
    (the "all-reduce the combined output" of the hint, done at unshard).
  - Capacity is one 512-token tile; the few tokens of experts loaded past
    C (~1% of token-expert pairs for balanced routing) are computed
    exactly on the host, which is both faster (25% less padded device
    work than capacity 640) and more accurate.

Matmuls run in float16 (exact in the PE's e10m11 internal format; ~3e-4
end-to-end rel err). fp16 operands halve HBM weight traffic vs fp32
(23 MB/core, ~65 us at 358 GB/s) so the DMA stream stays well under the
~143 us PE streaming floor, and enable FWL fast weight loads. (fp8
would double PE throughput but measures 6.5% end-to-end rel err --
e4m3's ~2.4%/operand quantization noise does not average out over the
reduction -- far above the 2e-2 gate; residual-corrected fp8 needs 3
matmul passes = 1.5x fp16 cost. fp16 is the floor.)

Head/tail scheduling (measured on HW, per-core):
  - Framework preamble ~6.5-8.0 us before user code; first DMA payload
    ~1.5 us after the first issue; DMA rate ~250 GB/s aggregate before
    the clock boost, ~420 after (~210 cap per transfer, ~150 for
    2KB-line transfers); issue costs ~0.6 us of queue-engine time.
  - Clock boost (1.2 -> 2.4 GHz) fires after ~4.1 us of CONTINUOUS PE
    activity; a pre-boost idle gap resets the window (catastrophic:
    chunk-0 then runs at half clock), post-boost idles < ~1.5 us are
    safe. 11 warm-up matmuls guarantee the threshold.
  - Transfers on a queue are not FIFO: co-active transfers interleave
    packets, so a completion semaphore fires late whenever competitors
    are active. The Tile scheduler reorders ready instructions past
    blocked ones (and will hoist stage-B transfers into the critical
    early window), so transfer ordering is enforced with data
    dependencies: a gpsimd memset chain forms a ~0.52 us/tick clock,
    and tiny gpsimd "touch" copies (gate -> corner of a DMA's dest)
    release each held transfer at a chosen tick (gpsimd's queue is
    otherwise idle; touches on the scalar queue would block the silu
    activations behind them).
  - Wire plan: wk0 + two xT-lo k-pair pieces stream from t0 (the lo
    chains start eating at the first 256KB boundary ~11.9), xT-hi
    releases at tick 4 (lands ~14.3 just before the g-hi chains), wk1
    at tick 6, everything else at tick 11. The m=0 chunk is consumed
    as g/u k-group chains matching arrival order.
  - The last stage-B chunk is split into shrinking token spans so the
    serial tail after the final matmul is only a 128-token mul + DMA.
"""

import numpy as np

E, TOPK, H, I = 8, 2, 1024, 3584
P = 128
NCORES = 8
KO = H // P          # 8 k-subtiles of the hidden dim
MI = I // P          # 28 chunks of the intermediate dim
MH = H // P          # 8 chunks of the output hidden dim


def _chunks_ge256(c):
    """Split c (multiple of 128, >=512) into pieces <=512, each >=256
    (float32r runs 1 cycle/row only for moving dims >= 256)."""
    out = []
    rem = c
    while rem > 512:
        take = 512 if rem - 512 >= 256 or rem == 1024 else rem - 256
        if rem - take != 0 and rem - take < 256:
            take = rem - 256
        out.append(take)
        rem -= take
    out.append(rem)
    assert sum(out) == c and all(256 <= t <= 512 for t in out), (c, out)
    return out


def _build_program(C, chunks):
    import concourse.bacc as bacc
    import concourse.mybir as mybir
    from concourse.tile import TileContext

    DT = mybir.dt.float16
    F32 = mybir.dt.float32
    AF = mybir.ActivationFunctionType

    nc = bacc.Bacc("TRN2", target_bir_lowering=False, debug=False,
                   num_devices=NCORES)
    xT_d = nc.dram_tensor("xT", [P, KO, C], DT, kind="ExternalInput")
    # w13 per-chunk layout is k-half-major: [m, p_h, khalf, g/u, k%4, p_i],
    # so a k-half of a chunk (both g and u) is one contiguous 4KB-per-
    # partition-line DMA (4KB lines run ~420 GB/s vs ~150 GB/s for 1KB).
    w13_d = nc.dram_tensor("w13", [MI, P, 2, 2, KO // 2, P], DT,
                           kind="ExternalInput")
    w2_d = nc.dram_tensor("w2r", [MH, P, MI, P], DT, kind="ExternalInput")
    wgt_d = nc.dram_tensor("wgt", [P, C], DT, kind="ExternalInput")
    out_d = nc.dram_tensor("outT", [H, C], DT, kind="ExternalOutput")

    with TileContext(nc) as tc:
        with tc.tile_pool(name="const", bufs=1) as constp, \
             tc.tile_pool(name="wpool", bufs=5) as wpool, \
             tc.tile_pool(name="w2pool", bufs=3) as w2pool, \
             tc.tile_pool(name="hpool", bufs=1) as hpool, \
             tc.tile_pool(name="tmp", bufs=3) as tmpp, \
             tc.tile_pool(name="outp", bufs=3) as outp, \
             tc.tile_pool(name="psA", bufs=2, space="PSUM") as psA, \
             tc.tile_pool(name="psB", bufs=2, space="PSUM") as psB:

            # Timing model (measured): user code entry ~6.5-7.2 us
            # (framework preamble), first DMA packet ~1.5 us after the
            # first issue, DMA rate ~250 GB/s before the clock boost and
            # ~390 GB/s after, boost after ~4.1 us of CONTINUOUS PE
            # activity (an idle gap before the boost resets the window;
            # a post-boost idle does not re-throttle). Transfers on one
            # queue are NOT strictly FIFO: packets of co-active transfers
            # interleave, so a transfer's completion semaphore fires late
            # whenever anything else is active -- and the Tile scheduler
            # reorders ready instructions past blocked ones, so ordering
            # must be enforced with real data dependencies. Head plan:
            # wk0 (one 4KB-line transfer) then xT-lo stream back-to-back
            # with ZERO competition (xT-lo done ~12.1); every other early
            # DMA (xT-hi, wk1..wk4) is held back by a "touch" write into
            # its destination that depends on a serialized gpsimd memset
            # chain ending ~11.3, then they stream one after another at
            # the boosted rate, each landing just before its consumer.
            half = KO // 2
            wk0 = wpool.tile([P, 2, 2, half, P], DT, tag="wk", name="wk0")
            xT = constp.tile([P, KO, C], DT)
            # (Both HWDGE queues share one DRAM channel -- measured: the
            # same ~250 GB/s pre-boost / ~420 post-boost aggregate and
            # ~210 per-transfer cap apply whether transfers ride one
            # queue or two -- so everything stays on the sync queue
            # where issue order is controllable.)
            # wk0 ships as its two k-halves and xT-lo as two k-pairs,
            # all issued at t0: the pre-boost window (~8.7-11.4)
            # delivers ~675KB total however it is split (line size is
            # irrelevant under the aggregate cap), but fine slicing lets
            # the g/u chains start at the first wk0-lo+xT-k01 boundary
            # (~11.0) instead of waiting for the full 512KB wk0. The
            # chains' per-slice RAW deps gate each matmul on exactly the
            # piece it reads.
            nc.sync.dma_start(wk0[:, 0], w13_d[0, :, 0])
            nc.sync.dma_start(xT[:, :2], xT_d[:, :2])
            nc.sync.dma_start(xT[:, 2:half], xT_d[:, 2:half])
            nc.sync.dma_start(wk0[:, 1], w13_d[0, :, 1])
            nc.sync.dma_start(xT[:, half:], xT_d[:, half:])

            # PE warm-up: dummy fp16 matmuls on a zeroed tile while the
            # first operands stream in; 10 warms x 427 ns (slow clock)
            # guarantee the ~4.1 us continuous-activity boost threshold.
            warm = tmpp.tile([P, 512], DT, tag="warm", name="warm")
            nc.gpsimd.memset(warm[:], 0.0)
            psw = psB.tile([P, 512], F32, tag="psb0", name="psw")

            def warm_mms(n):
                for _ in range(n):
                    nc.tensor.matmul(psw[:], warm[:, :P], warm[:],
                                     start=True, stop=True)

            warm_mms(11)
            # Delay gate: serialized [P,512] gpsimd memsets (~0.52 us
            # each on the slow clock, WAW on the same tile) form a clock;
            # a touch emitted after memset #k waits for k completions.
            # EVERY DMA that would otherwise be ready in the early window
            # must be gated, or the scheduler hoists it into the xT-lo
            # stream (it will even hoist stage-B w2/wgt transfers), and
            # the releases must be STAGGERED so each transfer streams
            # near-solo, landing just before its consumer needs it:
            # xT-hi after #6 (~10.7, lands ~13.2, needed 13.9), wk1
            # after #7 (~11.2, lands ~14.5, needed 15.4), the rest after
            # #8 (~11.8, plenty of slack).
            gate = tmpp.tile([P, 512], DT, tag="gate", name="gate")

            def touch(dst, src):
                # 16B copy: reads 8 elements of `src` (RAW on its last
                # writer) and writes a corner of the DMA's destination,
                # so the DMA issue (WAW) waits for it. On GPSIMD: that
                # queue is idle after the gate memsets, so its in-order
                # blocking can't delay real work (on the scalar queue
                # these block the stage-A silu activations).
                nc.gpsimd.tensor_copy(dst, src)

            # Tick clock: gpsimd memset #k completes ~entry + 0.52*(k+1)
            # us (the 523 ns memset duration is boost-invariant). A touch
            # emitted after k memsets releases its DMA at tick k; ticks
            # are spaced so each transfer runs SOLO (concurrent transfers
            # collapse the queue to ~200-225 GB/s total vs ~390 solo) and
            # lands just before its consumer.
            tick = [0]

            def tick_to(k):
                while tick[0] < k:
                    nc.gpsimd.memset(gate[:], 0.0)
                    tick[0] += 1

            # Solo transfers run ~175-205 GB/s; 2+ concurrent transfers
            # aggregate ~390-430 (parallel DMA engines), pre-boost ~250
            # total. So: run PAIRS matched to consumption order. Pair 1
            # (wk0 + xT-lo, both needed by the lo chains) streams from
            # t0, done ~12.4. Pair 2 (xT-hi + wk1) releases at tick 6,
            # mostly after pair 1, done ~14.3 (g-hi needs ~14.1, chunk-1
            # ~15.9). Everything else releases at tick 13 (~14.0) and
            # shares the wire 7-way -- all have >5 us of slack.
            # xT-hi releases at tick 4 (~9.6): it trickles as a 3rd
            # stream behind pair-1 (slipping xT-lo slightly) but gets a
            # wide window, landing ~14.3 where the g-hi chains need it.
            # A lone 512KB transfer released later is capped at ~210
            # GB/s and cannot fit the 1.7 us gap between the lo chains'
            # end and the hi chains' need (k-pair splits don't help:
            # their 2KB lines cap at ~150 GB/s each).
            wk_pre = {}
            tick_to(9)
            wkm = wpool.tile([P, 2, 2, half, P], DT, tag="wk", name="wk")
            touch(wkm[:1, 0, 0, 0, :8], gate[:1, :8])
            nc.sync.dma_start(wkm[:], w13_d[1])
            wk_pre[1] = wkm
            tick_to(12)
            for m in range(2, 5):
                wkm = wpool.tile([P, 2, 2, half, P], DT, tag="wk",
                                 name="wk")
                touch(wkm[:1, 0, 0, 0, :8], gate[:1, :8])
                nc.sync.dma_start(wkm[:], w13_d[m])
                wk_pre[m] = wkm
            w2_pre = {}
            for mh in range(3):
                w2m = w2pool.tile([P, MI, P], DT, tag="w2c", name="w2c")
                touch(w2m[:1, 0, :8], gate[:1, :8])
                nc.sync.dma_start(w2m[:], w2_d[mh])
                w2_pre[mh] = w2m
            hT = hpool.tile([P, MI, C], DT)

            # ---- stage A: hT[i, t] = silu(g) * u over all I-chunks ----
            def gchain(ps, wkt, widx, ks, tn, start, stop):
                for j, k in enumerate(ks):
                    nc.tensor.matmul(ps, wkt[:, k // half, widx, k % half],
                                     xT[:, k, :tn],
                                     start=(start and j == 0),
                                     stop=(stop and j == len(ks) - 1))

            def finish_chunk(m, psg, psu, t0, tn):
                sg = tmpp.tile([P, 512], F32, tag="sg", name="sg")[:, :tn]
                nc.scalar.activation(sg, psg, AF.Silu)
                nc.vector.tensor_mul(hT[:, m, t0:t0 + tn], sg, psu)

            if chunks == [C]:
                # m=0 scheduled as g/u k-half chains matching the DMA
                # arrival order: the lo chains gate on wk0+xT-lo (~12.4),
                # the hi chains on xT-hi (~14.3) which lands while the lo
                # chains (~1.7 us) are still streaming.
                hi = range(half, KO)
                psg0 = psA.tile([P, 512], F32, tag="psg", name="psg")[:, :C]
                psu0 = psA.tile([P, 512], F32, tag="psu", name="psu")[:, :C]
                gchain(psg0, wk0, 0, [0, 1], C, start=True, stop=False)
                gchain(psu0, wk0, 1, [0, 1], C, start=True, stop=False)
                gchain(psg0, wk0, 0, [2, 3], C, start=False, stop=False)
                gchain(psu0, wk0, 1, [2, 3], C, start=False, stop=False)
                gchain(psg0, wk0, 0, hi, C, start=False, stop=True)
                gchain(psu0, wk0, 1, hi, C, start=False, stop=True)
                finish_chunk(0, psg0, psu0, 0, C)
                start_m = 1
            else:
                start_m = 0
            for m in range(start_m, MI):
                if m == 0:
                    wk = wk0
                elif m in wk_pre:
                    wk = wk_pre[m]
                else:
                    wk = wpool.tile([P, 2, 2, half, P], DT, tag="wk",
                                    name="wk")
                    nc.sync.dma_start(wk[:], w13_d[m])
                t0 = 0
                for tn in chunks:
                    psg = psA.tile([P, 512], F32, tag="psg", name="psg")[:, :tn]
                    psu = psA.tile([P, 512], F32, tag="psu", name="psu")[:, :tn]
                    for k in range(KO):
                        nc.tensor.matmul(psg, wk[:, k // half, 0, k % half],
                                         xT[:, k, t0:t0 + tn],
                                         start=(k == 0), stop=(k == KO - 1))
                    for k in range(KO):
                        nc.tensor.matmul(psu, wk[:, k // half, 1, k % half],
                                         xT[:, k, t0:t0 + tn],
                                         start=(k == 0), stop=(k == KO - 1))
                    finish_chunk(m, psg, psu, t0, tn)
                    t0 += tn

            # ---- stage B: outT[h, t] = sum_i w2T[i, h] * hT[i, t], * wgt ----
            # wgt is touch-gated like the early weight DMAs: the
            # scheduler would otherwise hoist it into the critical early
            # window that anchors the matmul stream's end.
            wgt = constp.tile([P, C], DT)
            nc.gpsimd.tensor_copy(wgt[:1, :8], gate[:1, :8])
            nc.sync.dma_start(wgt[:], wgt_d[:])
            for mh in range(MH):
                if mh in w2_pre:
                    w2c = w2_pre[mh]
                else:
                    w2c = w2pool.tile([P, MI, P], DT, tag="w2c", name="w2c")
                    nc.sync.dma_start(w2c[:], w2_d[mh])
                outsb = outp.tile([P, C], DT, tag="outsb", name="outsb")
                # Last H-chunk is split into shrinking spans so its wgt-mul
                # + output DMA overlap the later spans' matmuls, shortening
                # the serial tail chain after the final matmul.
                if mh == MH - 1 and chunks == [512]:
                    spans = [(0, 256), (256, 384), (384, 512)]
                else:
                    spans = []
                    t0 = 0
                    for tn in chunks:
                        spans.append((t0, t0 + tn))
                        t0 += tn
                for j, (t0, t1) in enumerate(spans):
                    psb = psB.tile([P, 512], F32, tag=f"psb{j % 2}",
                                   name=f"psb{j % 2}")[:, :t1 - t0]
                    for k in range(MI):
                        nc.tensor.matmul(psb, w2c[:, k], hT[:, k, t0:t1],
                                         start=(k == 0), stop=(k == MI - 1))
                    nc.vector.tensor_mul(outsb[:, t0:t1], psb,
                                         wgt[:, t0:t1])
                    nc.sync.dma_start(out_d[mh * P:(mh + 1) * P, t0:t1],
                                      outsb[:, t0:t1])
    nc.compile()
    return nc


def kernel(hidden_states, w_gate, w1, w3, w2, _trace=False):
    from concourse.bass_utils import run_bass_kernel_spmd

    B, S, Hd = hidden_states.shape
    x = np.ascontiguousarray(hidden_states, dtype=np.float32).reshape(-1, Hd)
    T = x.shape[0]

    # ---- routing (host): logits -> softmax -> top-2 -> renormalize ----
    logits = x @ np.asarray(w_gate, dtype=np.float32).T
    zmax = logits.max(-1, keepdims=True)
    ez = np.exp(logits - zmax)
    probs = ez / ez.sum(-1, keepdims=True)
    top2 = np.argpartition(-probs, TOPK - 1, axis=-1)[:, :TOPK]
    topw = np.take_along_axis(probs, top2, -1)
    topw = topw / topw.sum(-1, keepdims=True)

    idx_list, wv_list = [], []
    for eid in range(E):
        tok, kk = np.nonzero(top2 == eid)
        idx_list.append(tok)
        wv_list.append(topw[tok, kk].astype(np.float32))
    maxn = max(len(ix) for ix in idx_list)
    # capacity: one 512 tile when loads are near-balanced (overflow runs on
    # host); scale up for pathological routing.
    C = 512 if maxn <= 640 else max(((maxn + P - 1) // P) * P, 512)
    chunks = _chunks_ge256(C)

    nc = _build_program(C, chunks)

    w1 = np.asarray(w1, dtype=np.float32)
    w3 = np.asarray(w3, dtype=np.float32)
    w2 = np.asarray(w2, dtype=np.float32)

    in_maps = []
    for eid in range(E):
        ix, wv = idx_list[eid][:C], wv_list[eid][:C]
        n = len(ix)
        xg = np.zeros((C, Hd), np.float16)
        xg[:n] = x[ix]
        xTr = np.ascontiguousarray(xg.T.reshape(KO, P, C).transpose(1, 0, 2))
        # [MI, P_h, khalf, g/u, k%4, P_i]: a k-half of a chunk (g and u) is
        # contiguous per partition so its DMA uses 4KB lines.
        w1p = (w1[eid].astype(np.float16).reshape(MI, P, KO, P)
               .transpose(0, 3, 2, 1).reshape(MI, P, 2, KO // 2, P))
        w3p = (w3[eid].astype(np.float16).reshape(MI, P, KO, P)
               .transpose(0, 3, 2, 1).reshape(MI, P, 2, KO // 2, P))
        w13 = np.ascontiguousarray(np.stack([w1p, w3p], axis=3))
        w2p = np.ascontiguousarray(
            w2[eid].astype(np.float16).reshape(MH, P, MI, P).transpose(0, 3, 2, 1))
        wg = np.zeros((C,), np.float32)
        wg[:n] = wv
        wgt_rep = np.ascontiguousarray(
            np.broadcast_to(wg.astype(np.float16), (P, C)))
        in_maps.append({"xT": xTr, "w13": w13, "w2r": w2p, "wgt": wgt_rep})

    res = run_bass_kernel_spmd(nc, in_maps, core_ids=list(range(NCORES)),
                               trace=_trace)

    y = np.zeros((T, Hd), np.float32)
    for eid in range(E):
        ix = idx_list[eid][:C]
        outT = np.asarray(res.results[eid]["outT"], np.float32)   # [H, C]
        y[ix] += outT[:, :len(ix)].T
        # overflow tokens past capacity: exact host compute (tiny)
        ov_ix, ov_wv = idx_list[eid][C:], wv_list[eid][C:]
        if len(ov_ix):
            xs = x[ov_ix]
            g = xs @ w1[eid].T
            u = xs @ w3[eid].T
            h = (g / (1.0 + np.exp(-g))) * u
            y[ov_ix] += ov_wv[:, None] * (h @ w2[eid].T)
    y = y.reshape(B, S, Hd)
    if _trace:
        return y, res
    return y



# revision 43
# speedup vs baseline: 1.0013x; 1.0013x over previous
"""Mixtral MoE block (E=8 experts, top-2, H=1024, I=3584) on 8 Trainium2 cores.

Strategy (expert-parallel, per sharding hint):
  - Host computes the router (logits -> softmax -> top-2 -> renormalized
    combine weights). This is 0.04% of the model FLOPs; all heavy compute
    (the expert SwiGLU MLPs, ~90 GFLOP) runs on the NeuronCores.
  - Tokens are gathered per expert on the host (dispatch); core e computes
    expert e's MLP over a capacity-C batch of its tokens:
        hT = silu(W1 xT) * (W3 xT)                      [I, C]
        outT = W2-col-slices^T @ hT, scaled by routing weight   [H, C]
    (token-transposed layouts throughout; zero on-device transposes)
  - Host scatter-adds the per-core weighted outputs back into [T, H]
    (the "all-reduce the combined output" of the hint, done at unshard).
  - Capacity is one 512-token tile; the few tokens of experts loaded past
    C (~1% of token-expert pairs for balanced routing) are computed
    exactly on the host, which is both faster (25% less padded device
    work than capacity 640) and more accurate.

Matmuls run in float16 (exact in the PE's e10m11 internal format; ~3e-4
end-to-end rel err). fp16 operands halve HBM weight traffic vs fp32
(23 MB/core, ~65 us at 358 GB/s) so the DMA stream stays well under the
~143 us PE streaming floor, and enable FWL fast weight loads. (fp8
would double PE throughput but measures 6.5% end-to-end rel err --
e4m3's ~2.4%/operand quantization noise does not average out over the
reduction -- far above the 2e-2 gate; residual-corrected fp8 needs 3
matmul passes = 1.5x fp16 cost. fp16 is the floor.)

Head/tail scheduling (measured on HW, per-core):
  - Framework preamble ~6.5-8.0 us before user code; first DMA payload
    ~1.5 us after the first issue; DMA rate ~250 GB/s aggregate before
    the clock boost, ~420 after (~210 cap per transfer, ~150 for
    2KB-line transfers); issue costs ~0.6 us of queue-engine time.
  - Clock boost (1.2 -> 2.4 GHz) fires after ~4.1 us of CONTINUOUS PE
    activity; a pre-boost idle gap resets the window (catastrophic:
    chunk-0 then runs at half clock), post-boost idles < ~1.5 us are
    safe. 11 warm-up matmuls guarantee the threshold.
  - Transfers on a queue are not FIFO: co-active transfers interleave
    packets, so a completion semaphore fires late whenever competitors
    are active. The Tile scheduler reorders ready instructions past
    blocked ones (and will hoist stage-B transfers into the critical
    early window), so transfer ordering is enforced with data
    dependencies: a gpsimd memset chain forms a ~0.52 us/tick clock,
    and tiny gpsimd "touch" copies (gate -> corner of a DMA's dest)
    release each held transfer at a chosen tick (gpsimd's queue is
    otherwise idle; touches on the scalar queue would block the silu
    activations behind them).
  - Wire plan: wk0 + two xT-lo k-pair pieces stream from t0 (the lo
    chains start eating at the first 256KB boundary ~11.9), xT-hi
    releases at tick 4 (lands ~14.3 just before the g-hi chains), wk1
    at tick 6, everything else at tick 11. The m=0 chunk is consumed
    as g/u k-group chains matching arrival order.
  - The last stage-B chunk is split into shrinking token spans so the
    serial tail after the final matmul is only a 128-token mul + DMA.
"""

import numpy as np

E, TOPK, H, I = 8, 2, 1024, 3584
P = 128
NCORES = 8
KO = H // P          # 8 k-subtiles of the hidden dim
MI = I // P          # 28 chunks of the intermediate dim
MH = H // P          # 8 chunks of the output hidden dim


def _chunks_ge256(c):
    """Split c (multiple of 128, >=512) into pieces <=512, each >=256
    (float32r runs 1 cycle/row only for moving dims >= 256)."""
    out = []
    rem = c
    while rem > 512:
        take = 512 if rem - 512 >= 256 or rem == 1024 else rem - 256
        if rem - take != 0 and rem - take < 256:
            take = rem - 256
        out.append(take)
        rem -= take
    out.append(rem)
    assert sum(out) == c and all(256 <= t <= 512 for t in out), (c, out)
    return out


def _build_program(C, chunks):
    import concourse.bacc as bacc
    import concourse.mybir as mybir
    from concourse.tile import TileContext

    DT = mybir.dt.float16
    F32 = mybir.dt.float32
    AF = mybir.ActivationFunctionType

    nc = bacc.Bacc("TRN2", target_bir_lowering=False, debug=False,
                   num_devices=NCORES)
    xT_d = nc.dram_tensor("xT", [P, KO, C], DT, kind="ExternalInput")
    # w13 per-chunk layout is k-half-major: [m, p_h, khalf, g/u, k%4, p_i],
    # so a k-half of a chunk (both g and u) is one contiguous 4KB-per-
    # partition-line DMA (4KB lines run ~420 GB/s vs ~150 GB/s for 1KB).
    w13_d = nc.dram_tensor("w13", [MI, P, 2, 2, KO // 2, P], DT,
                           kind="ExternalInput")
    w2_d = nc.dram_tensor("w2r", [MH, P, MI, P], DT, kind="ExternalInput")
    wgt_d = nc.dram_tensor("wgt", [P, C], DT, kind="ExternalInput")
    out_d = nc.dram_tensor("outT", [H, C], DT, kind="ExternalOutput")

    with TileContext(nc) as tc:
        with tc.tile_pool(name="const", bufs=1) as constp, \
             tc.tile_pool(name="wpool", bufs=5) as wpool, \
             tc.tile_pool(name="w2pool", bufs=3) as w2pool, \
             tc.tile_pool(name="hpool", bufs=1) as hpool, \
             tc.tile_pool(name="tmp", bufs=3) as tmpp, \
             tc.tile_pool(name="outp", bufs=3) as outp, \
             tc.tile_pool(name="psA", bufs=2, space="PSUM") as psA, \
             tc.tile_pool(name="psB", bufs=2, space="PSUM") as psB:

            # Timing model (measured): user code entry ~6.5-7.2 us
            # (framework preamble), first DMA packet ~1.5 us after the
            # first issue, DMA rate ~250 GB/s before the clock boost and
            # ~390 GB/s after, boost after ~4.1 us of CONTINUOUS PE
            # activity (an idle gap before the boost resets the window;
            # a post-boost idle does not re-throttle). Transfers on one
            # queue are NOT strictly FIFO: packets of co-active transfers
            # interleave, so a transfer's completion semaphore fires late
            # whenever anything else is active -- and the Tile scheduler
            # reorders ready instructions past blocked ones, so ordering
            # must be enforced with real data dependencies. Head plan:
            # wk0 (one 4KB-line transfer) then xT-lo stream back-to-back
            # with ZERO competition (xT-lo done ~12.1); every other early
            # DMA (xT-hi, wk1..wk4) is held back by a "touch" write into
            # its destination that depends on a serialized gpsimd memset
            # chain ending ~11.3, then they stream one after another at
            # the boosted rate, each landing just before its consumer.
            half = KO // 2
            wk0 = wpool.tile([P, 2, 2, half, P], DT, tag="wk", name="wk0")
            xT = constp.tile([P, KO, C], DT)
            # (Both HWDGE queues share one DRAM channel -- measured: the
            # same ~250 GB/s pre-boost / ~420 post-boost aggregate and
            # ~210 per-transfer cap apply whether transfers ride one
            # queue or two -- so everything stays on the sync queue
            # where issue order is controllable.)
            # wk0 ships as its two k-halves and xT-lo as two k-pairs,
            # all issued at t0: the pre-boost window (~8.7-11.4)
            # delivers ~675KB total however it is split (line size is
            # irrelevant under the aggregate cap), but fine slicing lets
            # the g/u chains start at the first wk0-lo+xT-k01 boundary
            # (~11.0) instead of waiting for the full 512KB wk0. The
            # chains' per-slice RAW deps gate each matmul on exactly the
            # piece it reads.
            nc.sync.dma_start(wk0[:, 0], w13_d[0, :, 0])
            nc.sync.dma_start(xT[:, :2], xT_d[:, :2])
            nc.sync.dma_start(xT[:, 2:half], xT_d[:, 2:half])
            nc.sync.dma_start(wk0[:, 1], w13_d[0, :, 1])

            # PE warm-up: dummy fp16 matmuls on a zeroed tile while the
            # first operands stream in; 10 warms x 427 ns (slow clock)
            # guarantee the ~4.1 us continuous-activity boost threshold.
            warm = tmpp.tile([P, 512], DT, tag="warm", name="warm")
            nc.gpsimd.memset(warm[:], 0.0)
            psw = psB.tile([P, 512], F32, tag="psb0", name="psw")

            def warm_mms(n):
                for _ in range(n):
                    nc.tensor.matmul(psw[:], warm[:, :P], warm[:],
                                     start=True, stop=True)

            warm_mms(11)
            # Delay gate: serialized [P,512] gpsimd memsets (~0.52 us
            # each on the slow clock, WAW on the same tile) form a clock;
            # a touch emitted after memset #k waits for k completions.
            # EVERY DMA that would otherwise be ready in the early window
            # must be gated, or the scheduler hoists it into the xT-lo
            # stream (it will even hoist stage-B w2/wgt transfers), and
            # the releases must be STAGGERED so each transfer streams
            # near-solo, landing just before its consumer needs it:
            # xT-hi after #6 (~10.7, lands ~13.2, needed 13.9), wk1
            # after #7 (~11.2, lands ~14.5, needed 15.4), the rest after
            # #8 (~11.8, plenty of slack).
            gate = tmpp.tile([P, 512], DT, tag="gate", name="gate")

            def touch(dst, src):
                # 16B copy: reads 8 elements of `src` (RAW on its last
                # writer) and writes a corner of the DMA's destination,
                # so the DMA issue (WAW) waits for it. On GPSIMD: that
                # queue is idle after the gate memsets, so its in-order
                # blocking can't delay real work (on the scalar queue
                # these block the stage-A silu activations).
                nc.gpsimd.tensor_copy(dst, src)

            # Tick clock: gpsimd memset #k completes ~entry + 0.52*(k+1)
            # us (the 523 ns memset duration is boost-invariant). A touch
            # emitted after k memsets releases its DMA at tick k; ticks
            # are spaced so each transfer runs SOLO (concurrent transfers
            # collapse the queue to ~200-225 GB/s total vs ~390 solo) and
            # lands just before its consumer.
            tick = [0]

            def tick_to(k):
                while tick[0] < k:
                    nc.gpsimd.memset(gate[:], 0.0)
                    tick[0] += 1

            # Solo transfers run ~175-205 GB/s; 2+ concurrent transfers
            # aggregate ~390-430 (parallel DMA engines), pre-boost ~250
            # total. So: run PAIRS matched to consumption order. Pair 1
            # (wk0 + xT-lo, both needed by the lo chains) streams from
            # t0, done ~12.4. Pair 2 (xT-hi + wk1) releases at tick 6,
            # mostly after pair 1, done ~14.3 (g-hi needs ~14.1, chunk-1
            # ~15.9). Everything else releases at tick 13 (~14.0) and
            # shares the wire 7-way -- all have >5 us of slack.
            # xT-hi releases at tick 4 (~9.6): it trickles as a 3rd
            # stream behind pair-1 (slipping xT-lo slightly) but gets a
            # wide window, landing ~14.3 where the g-hi chains need it.
            # A lone 512KB transfer released later is capped at ~210
            # GB/s and cannot fit the 1.7 us gap between the lo chains'
            # end and the hi chains' need (k-pair splits don't help:
            # their 2KB lines cap at ~150 GB/s each).
            tick_to(4)
            touch(xT[:1, half, :8], gate[:1, :8])
            nc.sync.dma_start(xT[:, half:], xT_d[:, half:])
            wk_pre = {}
            tick_to(6)
            wkm = wpool.tile([P, 2, 2, half, P], DT, tag="wk", name="wk")
            touch(wkm[:1, 0, 0, 0, :8], gate[:1, :8])
            nc.sync.dma_start(wkm[:], w13_d[1])
            wk_pre[1] = wkm
            tick_to(11)
            for m in range(2, 5):
                wkm = wpool.tile([P, 2, 2, half, P], DT, tag="wk",
                                 name="wk")
                touch(wkm[:1, 0, 0, 0, :8], gate[:1, :8])
                nc.sync.dma_start(wkm[:], w13_d[m])
                wk_pre[m] = wkm
            w2_pre = {}
            for mh in range(3):
                w2m = w2pool.tile([P, MI, P], DT, tag="w2c", name="w2c")
                touch(w2m[:1, 0, :8], gate[:1, :8])
                nc.sync.dma_start(w2m[:], w2_d[mh])
                w2_pre[mh] = w2m
            hT = hpool.tile([P, MI, C], DT)

            # ---- stage A: hT[i, t] = silu(g) * u over all I-chunks ----
            def gchain(ps, wkt, widx, ks, tn, start, stop):
                for j, k in enumerate(ks):
                    nc.tensor.matmul(ps, wkt[:, k // half, widx, k % half],
                                     xT[:, k, :tn],
                                     start=(start and j == 0),
                                     stop=(stop and j == len(ks) - 1))

            def finish_chunk(m, psg, psu, t0, tn):
                sg = tmpp.tile([P, 512], F32, tag="sg", name="sg")[:, :tn]
                nc.scalar.activation(sg, psg, AF.Silu)
                nc.vector.tensor_mul(hT[:, m, t0:t0 + tn], sg, psu)

            if chunks == [C]:
                # m=0 scheduled as g/u k-half chains matching the DMA
                # arrival order: the lo chains gate on wk0+xT-lo (~12.4),
                # the hi chains on xT-hi (~14.3) which lands while the lo
                # chains (~1.7 us) are still streaming.
                hi = range(half, KO)
                psg0 = psA.tile([P, 512], F32, tag="psg", name="psg")[:, :C]
                psu0 = psA.tile([P, 512], F32, tag="psu", name="psu")[:, :C]
                gchain(psg0, wk0, 0, [0, 1], C, start=True, stop=False)
                gchain(psu0, wk0, 1, [0, 1], C, start=True, stop=False)
                gchain(psg0, wk0, 0, [2, 3], C, start=False, stop=False)
                gchain(psu0, wk0, 1, [2, 3], C, start=False, stop=False)
                gchain(psg0, wk0, 0, hi, C, start=False, stop=True)
                gchain(psu0, wk0, 1, hi, C, start=False, stop=True)
                finish_chunk(0, psg0, psu0, 0, C)
                start_m = 1
            else:
                start_m = 0
            for m in range(start_m, MI):
                if m == 0:
                    wk = wk0
                elif m in wk_pre:
                    wk = wk_pre[m]
                else:
                    wk = wpool.tile([P, 2, 2, half, P], DT, tag="wk",
                                    name="wk")
                    nc.sync.dma_start(wk[:], w13_d[m])
                t0 = 0
                for tn in chunks:
                    psg = psA.tile([P, 512], F32, tag="psg", name="psg")[:, :tn]
                    psu = psA.tile([P, 512], F32, tag="psu", name="psu")[:, :tn]
                    for k in range(KO):
                        nc.tensor.matmul(psg, wk[:, k // half, 0, k % half],
                                         xT[:, k, t0:t0 + tn],
                                         start=(k == 0), stop=(k == KO - 1))
                    for k in range(KO):
                        nc.tensor.matmul(psu, wk[:, k // half, 1, k % half],
                                         xT[:, k, t0:t0 + tn],
                                         start=(k == 0), stop=(k == KO - 1))
                    finish_chunk(m, psg, psu, t0, tn)
                    t0 += tn

            # ---- stage B: outT[h, t] = sum_i w2T[i, h] * hT[i, t], * wgt ----
            # wgt is touch-gated like the early weight DMAs: the
            # scheduler would otherwise hoist it into the critical early
            # window that anchors the matmul stream's end.
            wgt = constp.tile([P, C], DT)
            nc.gpsimd.tensor_copy(wgt[:1, :8], gate[:1, :8])
            nc.sync.dma_start(wgt[:], wgt_d[:])
            for mh in range(MH):
                if mh in w2_pre:
                    w2c = w2_pre[mh]
                else:
                    w2c = w2pool.tile([P, MI, P], DT, tag="w2c", name="w2c")
                    nc.sync.dma_start(w2c[:], w2_d[mh])
                outsb = outp.tile([P, C], DT, tag="outsb", name="outsb")
                # Last H-chunk is split into shrinking spans so its wgt-mul
                # + output DMA overlap the later spans' matmuls, shortening
                # the serial tail chain after the final matmul.
                if mh == MH - 1 and chunks == [512]:
                    spans = [(0, 256), (256, 384), (384, 512)]
                else:
                    spans = []
                    t0 = 0
                    for tn in chunks:
                        spans.append((t0, t0 + tn))
                        t0 += tn
                for j, (t0, t1) in enumerate(spans):
                    psb = psB.tile([P, 512], F32, tag=f"psb{j % 2}",
                                   name=f"psb{j % 2}")[:, :t1 - t0]
                    for k in range(MI):
                        nc.tensor.matmul(psb, w2c[:, k], hT[:, k, t0:t1],
                                         start=(k == 0), stop=(k == MI - 1))
                    nc.vector.tensor_mul(outsb[:, t0:t1], psb,
                                         wgt[:, t0:t1])
                    nc.sync.dma_start(out_d[mh * P:(mh + 1) * P, t0:t1],
                                      outsb[:, t0:t1])
    nc.compile()
    return nc


def kernel(hidden_states, w_gate, w1, w3, w2, _trace=False):
    from concourse.bass_utils import run_bass_kernel_spmd

    B, S, Hd = hidden_states.shape
    x = np.ascontiguousarray(hidden_states, dtype=np.float32).reshape(-1, Hd)
    T = x.shape[0]

    # ---- routing (host): logits -> softmax -> top-2 -> renormalize ----
    logits = x @ np.asarray(w_gate, dtype=np.float32).T
    zmax = logits.max(-1, keepdims=True)
    ez = np.exp(logits - zmax)
    probs = ez / ez.sum(-1, keepdims=True)
    top2 = np.argpartition(-probs, TOPK - 1, axis=-1)[:, :TOPK]
    topw = np.take_along_axis(probs, top2, -1)
    topw = topw / topw.sum(-1, keepdims=True)

    idx_list, wv_list = [], []
    for eid in range(E):
        tok, kk = np.nonzero(top2 == eid)
        idx_list.append(tok)
        wv_list.append(topw[tok, kk].astype(np.float32))
    maxn = max(len(ix) for ix in idx_list)
    # capacity: one 512 tile when loads are near-balanced (overflow runs on
    # host); scale up for pathological routing.
    C = 512 if maxn <= 640 else max(((maxn + P - 1) // P) * P, 512)
    chunks = _chunks_ge256(C)

    nc = _build_program(C, chunks)

    w1 = np.asarray(w1, dtype=np.float32)
    w3 = np.asarray(w3, dtype=np.float32)
    w2 = np.asarray(w2, dtype=np.float32)

    in_maps = []
    for eid in range(E):
        ix, wv = idx_list[eid][:C], wv_list[eid][:C]
        n = len(ix)
        xg = np.zeros((C, Hd), np.float16)
        xg[:n] = x[ix]
        xTr = np.ascontiguousarray(xg.T.reshape(KO, P, C).transpose(1, 0, 2))
        # [MI, P_h, khalf, g/u, k%4, P_i]: a k-half of a chunk (g and u) is
        # contiguous per partition so its DMA uses 4KB lines.
        w1p = (w1[eid].astype(np.float16).reshape(MI, P, KO, P)
               .transpose(0, 3, 2, 1).reshape(MI, P, 2, KO // 2, P))
        w3p = (w3[eid].astype(np.float16).reshape(MI, P, KO, P)
               .transpose(0, 3, 2, 1).reshape(MI, P, 2, KO // 2, P))
        w13 = np.ascontiguousarray(np.stack([w1p, w3p], axis=3))
        w2p = np.ascontiguousarray(
            w2[eid].astype(np.float16).reshape(MH, P, MI, P).transpose(0, 3, 2, 1))
        wg = np.zeros((C,), np.float32)
        wg[:n] = wv
        wgt_rep = np.ascontiguousarray(
            np.broadcast_to(wg.astype(np.float16), (P, C)))
        in_maps.append({"xT": xTr, "w13": w13, "w2r": w2p, "wgt": wgt_rep})

    res = run_bass_kernel_spmd(nc, in_maps, core_ids=list(range(NCORES)),
                               trace=_trace)

    y = np.zeros((T, Hd), np.float32)
    for eid in range(E):
        ix = idx_list[eid][:C]
        outT = np.asarray(res.results[eid]["outT"], np.float32)   # [H, C]
        y[ix] += outT[:, :len(ix)].T
        # overflow tokens past capacity: exact host compute (tiny)
        ov_ix, ov_wv = idx_list[eid][C:], wv_list[eid][C:]
        if len(ov_ix):
            xs = x[ov_ix]
            g = xs @ w1[eid].T
            u = xs @ w3[eid].T
            h = (g / (1.0 + np.exp(-g))) * u
            y[ov_ix] += ov_wv[:, None] * (h @ w2[eid].T)
    y = y.reshape(B, S, Hd)
    if _trace:
        return y, res
    return y

